# revision 20
# baseline (speedup 1.0000x reference)
"""CausalGateUnit Trainium2 kernel.

Math (see reference):
  p_pre = q @ W_pre + b_pre ; p_haz = q @ W_haz + b_haz          [B,S,D]
  gates = sigmoid(q @ W_gate + b_gate)                           [B,S,2]
  sim_x = (p_x @ k^T) * (1/sqrt(D)), strictly-causal masked (j<i)
  score_x[i] = max_j<i sim_x[i,j]   (0 when no visible j, i.e. i==0)
  rs = [g_pre, score_pre, g_haz, score_haz]                      [B,S,4]
  out = relu(rs @ W_s1 + b_s1) @ W_s2 + b_s2                     [B,S,D]

Sharding over 8 cores: core = (b, r) with b = core//4, r = core%4.
Core (b, r) owns row tiles t = 4g + r (g = 0..7) of batch b — 1024 rows.
Slot g computes score chunks over columns [0, 512*(g+1)); every core does an
identical instruction stream (144 col-tiles of 128x128 score matmul), with the
causal diagonal handled by a per-core constant bias tile added via an
identity matmul into PSUM. k^T is replicated per batch.
"""

import sys

for _p in ("/opt/trn_rl_repo",):
    if _p not in sys.path:
        sys.path.insert(0, _p)

import numpy as np

B, S, D = 2, 4096, 512
NCORES = 8
P = 128          # partitions / row-tile size
NSLOT = 8        # row tiles per core
ROWS = NSLOT * P  # 1024 rows per core
D1 = 256         # MLP hidden
CHUNK = 512      # score column chunk
CONSTW = 3584    # packed small-constant tile width
KT = D // P      # 4 contraction tiles
NEG = -1.0e30

_PROGRAM_CACHE = {}


def _build_program(debug=False, with_bias=True):
    import concourse.bacc as bacc
    import concourse.mybir as mybir
    import concourse.tile as tile

    f32 = mybir.dt.float32
    f32r = mybir.dt.float32r
    bf16 = mybir.dt.bfloat16
    AX = mybir.AxisListType
    MAX = mybir.AluOpType.max
    ACT = mybir.ActivationFunctionType

    nc = bacc.Bacc()

    qT_d = nc.declare_dram_parameter("qT", [D, ROWS], bf16, isOutput=False)
    kT_d = nc.declare_dram_parameter("kT", [D, S], bf16, isOutput=False)
    Wp_d = nc.declare_dram_parameter("Wp", [D, D], bf16, isOutput=False)
    Wh_d = nc.declare_dram_parameter("Wh", [D, D], bf16, isOutput=False)
    Wg_d = nc.declare_dram_parameter("Wg", [D, 2], bf16, isOutput=False)
    Ws2_d = nc.declare_dram_parameter("Ws2", [D1, D], bf16, isOutput=False)
    cn_d = nc.declare_dram_parameter("consts", [P, CONSTW], bf16, isOutput=False)
    cb_d = nc.declare_dram_parameter("cbf", [P, P + CHUNK], bf16, isOutput=False)
    out_d = nc.declare_dram_parameter("out", [ROWS, D], f32, isOutput=True)
    dbg_d = None
    if debug:
        dbg_d = nc.declare_dram_parameter("dbg_rsT", [5, ROWS], bf16, isOutput=True)
        dbg_pT = nc.declare_dram_parameter("dbg_pT", [P, KT, ROWS], bf16, isOutput=True)
        dbg_h1 = nc.declare_dram_parameter("dbg_h1", [P, 2, ROWS], bf16, isOutput=True)

    def r32(ap):
        return ap.bitcast(f32r) if ap.dtype == f32 else ap

    with tile.TileContext(nc) as tc:
        with (
            tc.tile_pool(name="const", bufs=1) as const,
            tc.tile_pool(name="scpart", bufs=4) as spool,
            tc.tile_pool(name="scfin", bufs=4) as fpool,
            tc.tile_pool(name="outs", bufs=3) as opool,
        ):
            kT_sb = const.tile([P, KT, S], bf16)
            qT_sb = const.tile([P, KT, ROWS], bf16)
            Wp_sb = const.tile([P, KT, D], bf16)
            Wh_sb = const.tile([P, KT, D], bf16)
            Wg_sb = const.tile([P, KT, 2], bf16)
            Ws2_sb = const.tile([P, 2, D], bf16)
            consts_sb = const.tile([P, CONSTW], bf16)
            pTp_sb = const.tile([P, KT, ROWS], bf16)
            pTh_sb = const.tile([P, KT, ROWS], bf16)
            h1T_sb = const.tile([P, 2, ROWS], bf16)
            rsT = const.tile([5, ROWS], bf16)
            cbf_sb = const.tile([P, P + CHUNK], bf16)
            ident = cbf_sb[:, 0:P]
            Cm_sb = cbf_sb[:, P : P + CHUNK]
            ones = consts_sb[0:1, 640:1664]
            Ws1_sb = consts_sb[0:5, 1664:1920]
            bp_sb = consts_sb[0:1, 1920:2432]
            bh_sb = consts_sb[0:1, 2432:2944]
            bs2_sb = consts_sb[0:1, 2944:3456]
            bg_sb = consts_sb[0:1, 3456:3458]

            # --- constant loads ---
            # qT/Wp/Wh first: phase A can start as soon as they land
            qT_r = qT_d[:, :].rearrange("(t p) n -> p t n", p=P)
            nc.sync.dma_start(out=qT_sb[:, :, 0:CHUNK], in_=qT_r[:, :, 0:CHUNK])
            nc.sync.dma_start(
                out=Wp_sb, in_=Wp_d[:, :].rearrange("(t p) n -> p t n", p=P)
            )
            nc.sync.dma_start(out=qT_sb[:, :, CHUNK:ROWS], in_=qT_r[:, :, CHUNK:ROWS])
            nc.sync.dma_start(
                out=Wh_sb, in_=Wh_d[:, :].rearrange("(t p) n -> p t n", p=P)
            )
            nc.sync.dma_start(out=consts_sb, in_=cn_d[:, :])
            nc.sync.dma_start(out=cbf_sb, in_=cb_d[:, :])
            nc.sync.dma_start(
                out=Wg_sb, in_=Wg_d[:, :].rearrange("(t p) n -> p t n", p=P)
            )
            nc.sync.dma_start(
                out=Ws2_sb, in_=Ws2_d[:, :].rearrange("(t p) n -> p t n", p=P)
            )
            # kT split by column chunk so slot g only waits on chunks <= g
            kT_r = kT_d[:, :].rearrange("(t p) n -> p t n", p=P)
            for c in range(S // CHUNK):
                cs = slice(c * CHUNK, (c + 1) * CHUNK)
                nc.sync.dma_start(out=kT_sb[:, :, cs], in_=kT_r[:, :, cs])

            # compute engines can't start at partition 4; DMA can
            nc.gpsimd.dma_start(out=rsT[4:5, :], in_=ones[0:1, :])

            # --- phase A: pT = (W^T qT) + b, gates ---
            # psX (2 banks) stays open through B for gate + MLP psums
            psX = tc.tile_pool(name="psX", bufs=2, space="PSUM")
            psXp = psX.__enter__()
            with tc.tile_pool(name="psA", bufs=3, space="PSUM") as psA:
                for n in range(ROWS // CHUNK):
                    ns = slice(n * CHUNK, (n + 1) * CHUNK)
                    for W_sb, b_sb, pT_sb in (
                        (Wp_sb, bp_sb, pTp_sb),
                        (Wh_sb, bh_sb, pTh_sb),
                    ):
                        for m in range(KT):
                            ms = slice(m * P, (m + 1) * P)
                            ps = psA.tile([P, CHUNK], f32, tag="pt")
                            for kt in range(KT):
                                nc.tensor.matmul(
                                    ps,
                                    lhsT=r32(W_sb[:, kt, ms]),
                                    rhs=r32(qT_sb[:, kt, ns]),
                                    start=(kt == 0),
                                    stop=(not with_bias and kt == KT - 1),
                                )
                            if with_bias:
                                nc.tensor.matmul(
                                    ps,
                                    lhsT=r32(b_sb[0:1, ms]),
                                    rhs=r32(ones[0:1, ns]),
                                    start=False,
                                    stop=True,
                                )
                            nc.scalar.copy(out=pT_sb[:, m, ns], in_=ps)

                    # gates -> rsT rows 0 (pre) and 2 (haz)
                    psg = psXp.tile([2, CHUNK], f32, tag="aux")
                    for kt in range(KT):
                        nc.tensor.matmul(
                            psg,
                            lhsT=r32(Wg_sb[:, kt, :]),
                            rhs=r32(qT_sb[:, kt, ns]),
                            start=(kt == 0),
                            stop=(not with_bias and kt == KT - 1),
                        )
                    if with_bias:
                        nc.tensor.matmul(
                            psg,
                            lhsT=r32(bg_sb[0:1, :]),
                            rhs=r32(ones[0:1, ns]),
                            start=False,
                            stop=True,
                        )
                    gt = fpool.tile([2, CHUNK], bf16, tag="gt")
                    nc.scalar.activation(out=gt, in_=psg, func=ACT.Sigmoid)
                    nc.gpsimd.dma_start(out=rsT[0:1, ns], in_=gt[0:1, :])
                    nc.gpsimd.dma_start(out=rsT[2:3, ns], in_=gt[1:2, :])

            # --- phase B: causal scores + row max, MLP fused per 4-slot block ---
            with tc.tile_pool(name="psB", bufs=3, space="PSUM") as psB:
                for g in (7, 6, 5, 4, 3, 2, 1, 0):
                    gs = slice(g * P, (g + 1) * P)
                    nch = g + 1
                    for pT_sb, ridx in ((pTp_sb, 1), (pTh_sb, 3)):
                        ngrp = (nch + 1) // 2
                        sct = fpool.tile([P, 1], bf16, tag="sct")
                        scp = None
                        if ngrp > 1:
                            scp = spool.tile([P, 4], f32, tag="scp")
                        for grp in range(ngrp):
                            c0 = grp * 2
                            c1 = min(c0 + 2, nch)
                            ps = psB.tile([P, 2, CHUNK], f32, tag="sc")
                            for c in range(c0, c1):
                                cs = slice(c * CHUNK, (c + 1) * CHUNK)
                                last = c == g
                                for kt in range(KT):
                                    nc.tensor.matmul(
                                        ps[:, c - c0, :],
                                        lhsT=r32(pT_sb[:, kt, gs]),
                                        rhs=r32(kT_sb[:, kt, cs]),
                                        start=(kt == 0),
                                        stop=(kt == KT - 1 and not last),
                                    )
                                if last:
                                    # += Cm (0 where j<i, -1e30 elsewhere)
                                    nc.tensor.matmul(
                                        ps[:, c - c0, :],
                                        lhsT=r32(ident),
                                        rhs=r32(Cm_sb),
                                        start=False,
                                        stop=True,
                                    )
                            red_out = sct if ngrp == 1 else scp[:, grp : grp + 1]
                            nc.vector.tensor_reduce(
                                out=red_out,
                                in_=ps[:, 0 : c1 - c0, :],
                                axis=AX.XY,
                                op=MAX,
                            )
                        if ngrp > 1:
                            nc.vector.tensor_reduce(
                                out=sct, in_=scp[:, 0:ngrp], axis=AX.X, op=MAX
                            )
                        # [128,1] -> [1,128] reorientation
                        nc.gpsimd.dma_start(out=rsT[ridx : ridx + 1, gs], in_=sct)

                    if g % 4 == 0:
                        # MLP for the completed 4-slot block
                        nb = g // 4
                        nbs = slice(nb * CHUNK, (nb + 1) * CHUNK)
                        for m in range(2):
                            ms = slice(m * P, (m + 1) * P)
                            phm = psXp.tile([P, CHUNK], f32, tag="aux", name="phm")
                            nc.tensor.matmul(
                                phm,
                                lhsT=r32(Ws1_sb[0:5, ms]),
                                rhs=r32(rsT[0:5, nbs]),
                                start=True,
                                stop=True,
                            )
                            nc.scalar.activation(
                                out=h1T_sb[:, m, nbs], in_=phm, func=ACT.Relu
                            )
                        for gg in range(4 * nb, 4 * nb + 4):
                            ggs = slice(gg * P, (gg + 1) * P)
                            ph = psXp.tile([P, D], f32, tag="aux", name="ph")
                            for m in range(2):
                                nc.tensor.matmul(
                                    ph,
                                    lhsT=r32(h1T_sb[:, m, ggs]),
                                    rhs=r32(Ws2_sb[:, m, :]),
                                    start=(m == 0),
                                    stop=(not with_bias and m == 1),
                                )
                            if with_bias:
                                nc.tensor.matmul(
                                    ph,
                                    lhsT=r32(ones[0:1, 0:P]),
                                    rhs=r32(bs2_sb[0:1, :]),
                                    start=False,
                                    stop=True,
                                )
                            ob = opool.tile([P, D], f32, tag="ob")
                            nc.any.tensor_copy(out=ob, in_=ph)
                            nc.sync.dma_start(out=out_d[ggs, :], in_=ob)
            psX.__exit__(None, None, None)
            if debug:
                nc.sync.dma_start(out=dbg_d[:, :], in_=rsT)
                nc.sync.dma_start(out=dbg_pT[:, :, :], in_=pTp_sb)
                nc.sync.dma_start(out=dbg_h1[:, :, :], in_=h1T_sb)

    nc.compile()
    return nc


def _get_program(debug=False, with_bias=True):
    key = ("nc_dbg" if debug else "nc") + ("_b" if with_bias else "")
    if key not in _PROGRAM_CACHE:
        _PROGRAM_CACHE[key] = _build_program(debug, with_bias)
    return _PROGRAM_CACHE[key]


def _row_index(r):
    # global row indices (within a batch) owned by core with residue r
    return np.concatenate(
        [np.arange(P) + P * (4 * g + r) for g in range(NSLOT)]
    )


def make_in_maps(q, k, W_pre, b_pre, W_haz, b_haz, W_gate, b_gate, W_s1, b_s1,
                 W_s2, b_s2):
    """Build the 8 per-core input dicts (host-side prep)."""
    import ml_dtypes

    bf = ml_dtypes.bfloat16
    scale = 1.0 / np.sqrt(np.float32(D))
    f = np.float32
    Wp = np.ascontiguousarray((W_pre * scale).astype(f).astype(bf))
    Wh = np.ascontiguousarray((W_haz * scale).astype(f).astype(bf))
    Wg = np.ascontiguousarray(W_gate.astype(f).astype(bf))
    Ws1 = np.concatenate([W_s1.astype(f), b_s1.astype(f).reshape(1, D1)], axis=0)
    Ws2 = np.ascontiguousarray(W_s2.astype(f).astype(bf))

    def packed_consts(r):
        c = np.zeros((P, CONSTW), f)
        c[0, 640:1664] = 1.0                                # ones
        c[0:5, 1664:1920] = Ws1                             # [5, 256] + b_s1
        c[0, 1920:2432] = (b_pre * scale).astype(f)
        c[0, 2432:2944] = (b_haz * scale).astype(f)
        c[0, 2944:3456] = b_s2.astype(f)
        c[0, 3456:3458] = b_gate.astype(f)
        return c.astype(bf)

    kTb = [np.ascontiguousarray(k[b].T.astype(f).astype(bf)) for b in range(B)]

    def packed_cbf(r):
        c = np.zeros((P, P + CHUNK), f)
        c[:, 0:P] = np.eye(P, dtype=f)
        pp, ff = np.mgrid[0:P, 0:CHUNK]
        c[:, P : P + CHUNK] = np.where(ff < P * r + pp, 0.0, NEG)
        return c.astype(bf)

    in_maps = []
    for core in range(NCORES):
        b, r = divmod(core, 4)
        rows = _row_index(r)
        qT = np.ascontiguousarray(q[b][rows, :].T.astype(f).astype(bf))
        in_maps.append(
            {
                "qT": qT,
                "kT": kTb[b],
                "Wp": Wp,
                "Wh": Wh,
                "Wg": Wg,
                "Ws2": Ws2,
                "consts": packed_consts(r),
                "cbf": packed_cbf(r),
            }
        )
    return in_maps


def assemble_output(results, q, W_gate, b_gate, W_s1, b_s1, W_s2, b_s2):
    out = np.empty((B, S, D), np.float32)
    for core in range(NCORES):
        b, r = divmod(core, 4)
        rows = _row_index(r)
        out[b][rows, :] = results[core]["out"]
    # row 0 of each batch: no visible keys -> score = 0 (exact host fixup)
    for b in range(B):
        g0 = 1.0 / (1.0 + np.exp(-(q[b, 0].astype(np.float64) @ W_gate + b_gate)))
        rs0 = np.array([g0[0], 0.0, g0[1], 0.0])
        h0 = np.maximum(rs0 @ W_s1 + b_s1, 0.0) @ W_s2 + b_s2
        out[b, 0, :] = h0.astype(np.float32)
    return out


def kernel(**inputs):
    from concourse.bass_utils import run_bass_kernel_spmd

    q = np.asarray(inputs["q"], np.float32)
    k = np.asarray(inputs["k"], np.float32)
    args = dict(
        q=q,
        k=k,
        W_pre=np.asarray(inputs["W_pre"], np.float32),
        b_pre=np.asarray(inputs["b_pre"], np.float32),
        W_haz=np.asarray(inputs["W_haz"], np.float32),
        b_haz=np.asarray(inputs["b_haz"], np.float32),
        W_gate=np.asarray(inputs["W_gate"], np.float32),
        b_gate=np.asarray(inputs["b_gate"], np.float32),
        W_s1=np.asarray(inputs["W_s1"], np.float32),
        b_s1=np.asarray(inputs["b_s1"], np.float32),
        W_s2=np.asarray(inputs["W_s2"], np.float32),
        b_s2=np.asarray(inputs["b_s2"], np.float32),
    )
    zero_bias = all(
        not np.any(args[b_]) for b_ in ("b_pre", "b_haz", "b_gate", "b_s1", "b_s2")
    )
    nc = _get_program(with_bias=not zero_bias)
    in_maps = make_in_maps(**args)
    res = run_bass_kernel_spmd(nc, in_maps, list(range(NCORES)))
    return assemble_output(
        res.results,
        q,
        args["W_gate"],
        args["b_gate"],
        args["W_s1"],
        args["b_s1"],
        args["W_s2"],
        args["b_s2"],
    )
